# revision 12
# baseline (speedup 1.0000x reference)
"""Trainium2 Bass kernel for nn_AlignModule (QAConv correlation + PAM).

Reference computation (B=32, C=512, H=24, W=8, hw=192, C8=64):
  xf = x.reshape(B, C, hw)
  score[g,p,n,m] = sum_c xf[g,c,m] * xf[p,c,n]          # [B,B,hw,hw]
  kernel_max[g,p,n] = max_m score[g,p,n,m]              # [B,B,hw]
  q = Wq @ xf[b] + bq; k = Wk @ xf[b] + bk              # [B,C8,hw]
  energy[b,m,n] = sum_q q[b,q,m] k[b,q,n]
  pos_max[b,m] = max_n energy[b,m,n]                    # [B,hw]
  out = concat([kernel_max, pos_max[None]], axis=0)     # [B+1,B,hw]

Sharding: data-parallel over g (and b for PAM) across 8 cores, 4 per core.
Each core receives the full x as a [C, B*hw] matrix (xT), rolled along the
column axis so its own 4 images occupy columns [0, 4*hw). The same SPMD
program then always reads its moving operand from columns [0, 768).

All QAConv matmuls are fp32r (FP22 multiply, fp32 accumulate) with moving
free dim 384 — the full-rate regime of the PE for 4-byte operands. The
stationary side packs the flattened (p, n) axis perfectly into 48
128-column blocks, so the PE runs at its fp32 roofline for this shape.
"""

import numpy as np

import concourse.bass as bass
import concourse.mybir as mybir
import concourse.tile as tile
from concourse import bacc
from concourse.bass_utils import run_bass_kernel_spmd
from concourse.masks import make_identity

B = 32
C = 512
HW = 192
C8 = 64
N_CORES = 8
GPC = B // N_CORES            # images per core (4)
FLAT = B * HW                 # flattened (p, n) axis (6144)
NJ = FLAT // 128              # stationary 128-column blocks (48)
NCC = 4                       # contraction chunks of 128 over C
GROLL = GPC * HW              # per-core roll step (768)
N_GP = GPC // 2               # g-pairs (moving operand batches of 2*HW=384)
JPCC = 6                      # j blocks per column-chunk of 768
F32 = mybir.dt.float32
F32R = mybir.dt.float32r
AX_X = mybir.AxisListType.X

_COMPILED = None


def _build():
    nc = bacc.Bacc("TRN2", target_bir_lowering=False, debug=False)

    xr = nc.dram_tensor("xr", [C, FLAT], F32R, kind="ExternalInput").ap()
    wq = nc.dram_tensor("wq", [C, C8], F32R, kind="ExternalInput").ap()
    wk = nc.dram_tensor("wk", [C, C8], F32R, kind="ExternalInput").ap()
    bq = nc.dram_tensor("bq", [C8, 1], F32, kind="ExternalInput").ap()
    bk = nc.dram_tensor("bk", [C8, 1], F32, kind="ExternalInput").ap()
    kmax = nc.dram_tensor("kmax", [GPC, FLAT], F32, kind="ExternalOutput").ap()
    pmax = nc.dram_tensor("pmax", [GPC, HW], F32, kind="ExternalOutput").ap()

    with tile.TileContext(nc) as tc:
        with (
            tc.tile_pool(name="xpool", bufs=1) as xpool,
            tc.tile_pool(name="wpool", bufs=1) as wpool,
            tc.tile_pool(name="respool", bufs=1) as respool,
            tc.tile_pool(name="outpool", bufs=1) as outpool,
            tc.tile_pool(name="qa_psum", bufs=6, space="PSUM") as qa_psum,
            tc.tile_pool(name="pam_psum", bufs=2, space="PSUM") as pam_psum,
        ):
            # ---- weights / biases ----
            wq_sb = wpool.tile([128, NCC, C8], F32R)
            nc.sync.dma_start(wq_sb[:], wq.rearrange("(co p) q -> p co q", p=128))
            wk_sb = wpool.tile([128, NCC, C8], F32R)
            nc.sync.dma_start(wk_sb[:], wk.rearrange("(co p) q -> p co q", p=128))
            bq_sb = wpool.tile([C8, 1], F32)
            nc.sync.dma_start(bq_sb[:], bq[:])
            bk_sb = wpool.tile([C8, 1], F32)
            nc.sync.dma_start(bk_sb[:], bk[:])

            ident = wpool.tile([128, 128], F32)
            make_identity(nc, ident[:])

            # ---- load x: 4 c-chunks x 8 column chunks of 768 ----
            xcb = [[None] * (FLAT // GROLL) for _ in range(NCC)]
            for cc in range(FLAT // GROLL):
                for c in range(NCC):
                    t = xpool.tile([128, GROLL], F32R, tag=f"x_{c}_{cc}")
                    nc.sync.dma_start(
                        t[:],
                        xr[c * 128:(c + 1) * 128, cc * GROLL:(cc + 1) * GROLL],
                    )
                    xcb[c][cc] = t

            # ---- PAM: q/k projections ----
            q_sb = wpool.tile([C8, GPC * HW], F32R)
            k_sb = wpool.tile([C8, GPC * HW], F32R)
            for gp in range(N_GP):
                q_ps = pam_psum.tile([C8, 2 * HW], F32, tag="pam", name=f"q_ps_{gp}")
                k_ps = pam_psum.tile([C8, 2 * HW], F32, tag="pam", name=f"k_ps_{gp}")
                for c in range(NCC):
                    rhs = xcb[c][0][:, gp * 2 * HW:(gp + 1) * 2 * HW]
                    nc.tensor.matmul(
                        q_ps[:], wq_sb[:, c, :], rhs,
                        start=(c == 0), stop=(c == NCC - 1),
                    )
                    nc.tensor.matmul(
                        k_ps[:], wk_sb[:, c, :], rhs,
                        start=(c == 0), stop=(c == NCC - 1),
                    )
                sl = slice(gp * 2 * HW, (gp + 1) * 2 * HW)
                nc.scalar.activation(
                    q_sb[:, sl], q_ps[:],
                    mybir.ActivationFunctionType.Identity, bias=bq_sb[:],
                )
                nc.scalar.activation(
                    k_sb[:, sl], k_ps[:],
                    mybir.ActivationFunctionType.Identity, bias=bk_sb[:],
                )

            # ---- PAM: energy + max over n ----
            # pam_sb[p, b, h]: h=0 -> pos_max[b, p] (m in [0,128));
            #                  h=1, p<64 -> pos_max[b, 128+p]
            pam_sb = respool.tile([128, GPC, 2], F32)
            for b in range(GPC):
                for mch, (m0, msz) in enumerate(((0, 128), (128, C8))):
                    e_ps = pam_psum.tile([128, HW], F32, tag="pam", name=f"e_ps_{b}_{mch}")
                    nc.tensor.matmul(
                        e_ps[:msz, :],
                        q_sb[:, b * HW + m0: b * HW + m0 + msz],
                        k_sb[:, b * HW:(b + 1) * HW],
                        start=True, stop=True,
                    )
                    nc.vector.reduce_max(
                        pam_sb[:msz, b, mch:mch + 1], e_ps[:msz, :], axis=AX_X,
                    )

            pam_t = outpool.tile([2 * GPC, 128], F32)
            tp = pam_psum.tile([128, 128], F32, tag="pam", name="tp_pam")
            nc.tensor.transpose(
                tp[:2 * GPC, :],
                pam_sb[:].rearrange("p b h -> p (b h)"),
                ident[:],
            )
            nc.scalar.copy(pam_t[:], tp[:2 * GPC, :])
            for b in range(GPC):
                nc.sync.dma_start(
                    pmax[b:b + 1, 0:128], pam_t[2 * b:2 * b + 1, :]
                )
                nc.sync.dma_start(
                    pmax[b:b + 1, 128:HW], pam_t[2 * b + 1:2 * b + 2, 0:C8]
                )

            # ---- QAConv: score blocks + max over m ----
            # res_sb[p, g, j] = kernel_max[g, j*128 + p] (rolled flat order)
            res_sb = respool.tile([128, GPC, NJ], F32)
            for j in range(NJ):
                cc, jl = divmod(j, JPCC)
                ps = [qa_psum.tile([128, 2, HW], F32, tag="qa_ps",
                                   name=f"qa_ps_{j}_{gp}")
                      for gp in range(N_GP)]
                for c in range(NCC):
                    lhsT = xcb[c][cc][:, jl * 128:(jl + 1) * 128]
                    for gp in range(N_GP):
                        rhs = xcb[c][0][:, gp * 2 * HW:(gp + 1) * 2 * HW]
                        nc.tensor.matmul(
                            ps[gp][:].rearrange("p a b -> p (a b)"),
                            lhsT,
                            rhs,
                            start=(c == 0),
                            stop=(c == NCC - 1),
                        )
                for gp in range(N_GP):
                    nc.vector.reduce_max(
                        res_sb[:, 2 * gp:2 * gp + 2, j], ps[gp][:], axis=AX_X,
                    )

            # ---- transpose results to output layout and store ----
            kout = outpool.tile([128, GPC, 128], F32)
            for g in range(GPC):
                tp = pam_psum.tile([128, 128], F32, tag="pam", name=f"tp_{g}")
                nc.tensor.transpose(tp[:NJ, :], res_sb[:, g, :], ident[:])
                nc.scalar.copy(kout[:NJ, g, :], tp[:NJ, :])
                nc.sync.dma_start(
                    kmax[g].rearrange("(j t) -> j t", t=128), kout[:NJ, g, :],
                )

    nc.compile()
    return nc


def kernel(x, Wq, bq, Wk, bk):
    global _COMPILED
    if _COMPILED is None:
        _COMPILED = _build()
    nc = _COMPILED

    x = np.ascontiguousarray(x, dtype=np.float32)
    xT = np.ascontiguousarray(
        x.reshape(B, C, HW).transpose(1, 0, 2).reshape(C, FLAT)
    )
    wqT = np.ascontiguousarray(np.asarray(Wq, np.float32).T)
    wkT = np.ascontiguousarray(np.asarray(Wk, np.float32).T)
    bq2 = np.ascontiguousarray(np.asarray(bq, np.float32).reshape(C8, 1))
    bk2 = np.ascontiguousarray(np.asarray(bk, np.float32).reshape(C8, 1))

    in_maps = [
        {
            "xr": np.ascontiguousarray(np.roll(xT, -i * GROLL, axis=1)),
            "wq": wqT,
            "wk": wkT,
            "bq": bq2,
            "bk": bk2,
        }
        for i in range(N_CORES)
    ]

    res = run_bass_kernel_spmd(nc, in_maps, core_ids=list(range(N_CORES)))

    kernel_max = np.empty((B, FLAT), np.float32)
    pos_max = np.empty((B, HW), np.float32)
    for i, r in enumerate(res.results):
        kernel_max[i * GPC:(i + 1) * GPC] = np.roll(r["kmax"], i * GROLL, axis=1)
        pos_max[i * GPC:(i + 1) * GPC] = r["pmax"]

    return np.concatenate(
        [kernel_max.reshape(B, B, HW), pos_max[None]], axis=0
    ).astype(np.float32)
